# revision 16
# baseline (speedup 1.0000x reference)
"""Causal self-attention (B=4, T=2048, C=1024, H=16) on 8 trn2 cores.

Sharding: core c -> batch b = c//2, head-group hg = c%2 (8 heads each).
Data parallel over B, tensor parallel over heads: each core computes
qkv for its 8 heads from its batch's x, runs causal attention, and a
row-parallel slice of the output projection. The host sums the two
head-group partials per batch and adds b_proj.

Per-core device program (all fp32):
  phase 1: V = x @ Wv + bv (natural [T, 512]); qk^T = (x @ Wqk + b)^T
           ([1024, T]) -- x^T supplied pre-transposed by the host.
  phase 2: per head-pair, per tq half [1024]: S^T tiles [tk=128, tq]
           via K=64 row-packed matmuls (two heads concurrently on the
           PE), exp on ACT (scale=1/8 fused), causal mask multiply on
           the diagonal 128x128 block only, then y^T += V_aug^T @ P^T
           with V_aug = [v | ones] so the softmax denominator falls out
           of the same matmul. Head B uses [ones | v] at PSUM base
           partition 63 so its y rows land on partitions 64..127
           (compute engines cannot cross partitions).
           Normalize y via reciprocal_approx_fast + partition_broadcast.
  phase 3: out_partial[T, 1024] = y @ Wp_slice from y^T tiles.
"""

import sys

for _p in ("/opt/trn_rl_repo",):
    if _p not in sys.path:
        sys.path.insert(0, _p)

from contextlib import ExitStack

import numpy as np

import concourse.bass as bass
import concourse.tile as tile
from concourse import bacc, mybir
from concourse.bass_utils import run_bass_kernel_spmd

F32 = mybir.dt.float32
T = 2048
C = 1024
NPAIR = 4          # head pairs per core
CS = NPAIR * 128   # per-core head width (8 heads x 64)
SCALE = 0.125      # 1/sqrt(64)
EXP = mybir.ActivationFunctionType.Exp


def _emit(tc, xt, wqk, bqk, wv, bv, wp, mask, out, dbg=None):
    nc = tc.nc
    with ExitStack() as top:
        consts = top.enter_context(tc.tile_pool(name="consts", bufs=1))
        mask_sb = consts.tile([128, 128], F32, tag="mask", name="mask_sb")
        nc.sync.dma_start(mask_sb, mask)
        bqk_sb = consts.tile([128, 8], F32, tag="bqk", name="bqk_sb")
        nc.gpsimd.dma_start(
            bqk_sb,
            bass.AP(tensor=bqk.tensor, offset=bqk.offset, ap=[[1, 128], [128, 8]]),
        )
        bv_bc = consts.tile([128, CS], F32, tag="bvbc", name="bv_bc")
        nc.gpsimd.dma_start(
            bv_bc,
            bass.AP(tensor=bv.tensor, offset=bv.offset, ap=[[0, 128], [1, CS]]),
        )
        ones_col = consts.tile([128, 64], F32, tag="ones", name="ones_col")
        nc.vector.memset(ones_col, 1.0)

        qkt_pool = top.enter_context(tc.tile_pool(name="qkt", bufs=1))
        qt = [qkt_pool.tile([128, T], F32, tag=f"qt{i}", name=f"qt{i}") for i in range(4)]
        kt = [qkt_pool.tile([128, T], F32, tag=f"kt{i}", name=f"kt{i}") for i in range(4)]
        v_pool = top.enter_context(tc.tile_pool(name="vpool", bufs=1))
        # per t-chunk: 4 pairs x [vA(64) | onesA | vB(64) | onesB]
        v_sb = [v_pool.tile([128, NPAIR * 130], F32, tag=f"v{t}", name=f"v{t}")
                for t in range(16)]
        with ExitStack() as ph1:
            xt_pool = ph1.enter_context(tc.tile_pool(name="xtp", bufs=1))
            xts = [xt_pool.tile([128, T], F32, tag=f"xt{k}", name=f"xt{k}")
                   for k in range(8)]
            for k in range(8):
                nc.sync.dma_start(xts[k], xt[k * 128:(k + 1) * 128, :])

            mm_ps = ph1.enter_context(tc.tile_pool(name="mmps", bufs=1, space="PSUM"))

            # ---- V = x @ Wv + bv, with the aug layout ----
            wv_pool = ph1.enter_context(tc.tile_pool(name="wvp", bufs=1))
            wvs = [wv_pool.tile([128, CS], F32, tag=f"wv{k}", name=f"wv{k}")
                   for k in range(8)]
            for k in range(8):
                nc.sync.dma_start(wvs[k], wv[k * 128:(k + 1) * 128, :])
            for t in range(16):
                pv = mm_ps.tile([128, CS], F32, tag="pv", bufs=2, name=f"pv{t}")
                for k in range(8):
                    nc.tensor.matmul(pv, xts[k][:, t * 128:(t + 1) * 128], wvs[k],
                                     start=(k == 0), stop=(k == 7))
                vt3 = v_sb[t].rearrange("p (h w) -> p h w", h=NPAIR)   # [128,4,130]
                pv3 = pv.rearrange("p (h w) -> p h w", h=NPAIR)        # [128,4,128]
                bv3 = bv_bc.rearrange("p (h w) -> p h w", h=NPAIR)
                nc.vector.tensor_add(vt3[:, :, 0:64], pv3[:, :, 0:64], bv3[:, :, 0:64])
                nc.vector.tensor_add(vt3[:, :, 65:129], pv3[:, :, 64:128],
                                     bv3[:, :, 64:128])
                nc.vector.memset(vt3[:, :, 64:65], 1.0)
                nc.vector.memset(vt3[:, :, 129:130], 1.0)

            # ---- qk^T = (x @ Wqk + b)^T : out chunks [128 ch, T] ----
            wqk_pool = ph1.enter_context(tc.tile_pool(name="wqkp", bufs=3))
            for m in (0, 4, 1, 5, 2, 6, 3, 7):
                dest = qt[m] if m < 4 else kt[m - 4]
                pq = [mm_ps.tile([128, 1024], F32, tag="pqk", bufs=2,
                                 name=f"pqk{m}_{h}") for h in range(2)]
                for k in range(8):
                    wc = wqk_pool.tile([128, 128], F32, tag="w", name=f"w{m}_{k}")
                    nc.sync.dma_start(wc, wqk[k * 128:(k + 1) * 128,
                                              m * 128:(m + 1) * 128])
                    for half in range(2):
                        for nn in range(2):
                            nc.tensor.matmul(
                                pq[half][:, nn * 512:(nn + 1) * 512], wc,
                                xts[k][:, half * 1024 + nn * 512:
                                       half * 1024 + (nn + 1) * 512],
                                start=(k == 0), stop=(k == 7))
                for half in range(2):
                    nc.vector.tensor_scalar_add(
                        dest[:, half * 1024:(half + 1) * 1024], pq[half],
                        bqk_sb[:, m:m + 1])

        if dbg is not None:
            for j in range(4):
                nc.sync.dma_start(dbg["qt"][j * 128:(j + 1) * 128, :], qt[j])
                nc.sync.dma_start(dbg["kt"][j * 128:(j + 1) * 128, :], kt[j])
            for t in range(16):
                nc.sync.dma_start(dbg["v"][t * 128:(t + 1) * 128, :], v_sb[t])

        # ---- attention ----
        y_pool = top.enter_context(tc.tile_pool(name="ypool", bufs=1))
        ys = [y_pool.tile([128, T], F32, tag=f"y{i}", name=f"y{i}") for i in range(4)]
        with ExitStack() as ph2:
            att_ps = ph2.enter_context(tc.tile_pool(name="attps", bufs=1, space="PSUM"))
            p_pool = ph2.enter_context(tc.tile_pool(name="ppool", bufs=2))
            d_pool = ph2.enter_context(tc.tile_pool(name="dpool", bufs=2))
            for jj in range(2):
                for hp in range(4):
                    y_a = att_ps.tile([65, 1024], F32, tag="ya", name=f"ya{jj}_{hp}")
                    y_b = att_ps.tile([65, 1024], F32, tag="yb", name=f"yb{jj}_{hp}")
                    for i in range(8 * jj + 8):
                        c0 = max(0, 128 * i - 1024 * jj)
                        s_a = att_ps.tile([128, 1024], F32, tag="sa",
                                          name=f"sa{jj}_{hp}_{i}")
                        s_b = att_ps.tile([128, 1024], F32, tag="sb",
                                          name=f"sb{jj}_{hp}_{i}")
                        p_a = p_pool.tile([128, 1024], F32, tag="pa",
                                          name=f"pa{jj}_{hp}_{i}")
                        p_b = p_pool.tile([128, 1024], F32, tag="pb",
                                          name=f"pb{jj}_{hp}_{i}")
                        pieces = ([(c0, 512), (512, 1024)] if c0 < 512
                                  else [(c0, 1024)])
                        ktile = slice(i * 128, (i + 1) * 128)
                        for (p0, p1) in pieces:
                            qcols = slice(jj * 1024 + p0, jj * 1024 + p1)
                            nc.tensor.matmul(s_a[:, p0:p1], kt[hp][0:64, ktile],
                                             qt[hp][0:64, qcols],
                                             start=True, stop=True)
                            nc.tensor.matmul(s_b[:, p0:p1], kt[hp][64:128, ktile],
                                             qt[hp][64:128, qcols],
                                             start=True, stop=True)
                        nc.scalar.activation(p_a[:, c0:1024], s_a[:, c0:1024],
                                             EXP, scale=SCALE)
                        nc.scalar.activation(p_b[:, c0:1024], s_b[:, c0:1024],
                                             EXP, scale=SCALE)
                        if i >= 8 * jj:
                            nc.vector.tensor_mul(p_a[:, c0:c0 + 128],
                                                 p_a[:, c0:c0 + 128], mask_sb)
                            nc.vector.tensor_mul(p_b[:, c0:c0 + 128],
                                                 p_b[:, c0:c0 + 128], mask_sb)
                        if dbg is not None and jj == 0 and hp == 0 and i == 0:
                            nc.sync.dma_start(dbg["p"][0:128, :], p_a)
                            nc.sync.dma_start(dbg["p"][128:256, :], p_b)
                        la = v_sb[i][:, 130 * hp: 130 * hp + 65]
                        lb = v_sb[i][:, 130 * hp + 65: 130 * hp + 130]
                        for (p0, p1) in pieces:
                            last_i = 8 * jj + (3 if p0 < 512 else 7)
                            nc.tensor.matmul(y_a[:, p0:p1], la, p_a[:, p0:p1],
                                             start=(i == 0), stop=(i == last_i))
                            nc.tensor.matmul(y_b[:, p0:p1], lb, p_b[:, p0:p1],
                                             start=(i == 0), stop=(i == last_i))
                    # evacuate y; denominators sit at PSUM partition 64 of each
                    da = d_pool.tile([65, 1024], F32, tag="da", name=f"da{jj}_{hp}")
                    db = d_pool.tile([65, 1024], F32, tag="db", name=f"db{jj}_{hp}")
                    rab = d_pool.tile([128, 1024], F32, tag="rab",
                                      name=f"rab{jj}_{hp}")
                    ybst = d_pool.tile([64, 1024], F32, tag="ybst",
                                       name=f"ybst{jj}_{hp}")
                    cols = slice(jj * 1024, (jj + 1) * 1024)
                    nc.vector.tensor_copy(da[64:65, :], y_a[64:65, :])
                    nc.vector.tensor_copy(db[64:65, :], y_b[64:65, :])
                    nc.vector.tensor_copy(ys[hp][0:64, cols], y_a[0:64, :])
                    nc.vector.tensor_copy(ybst, y_b[0:64, :])
                    # DMA is the only engine that can move data across partitions
                    nc.sync.dma_start(ys[hp][64:128, cols], ybst)
                    # PE outer-product broadcast: den rows -> all partitions
                    den_bc = att_ps.tile([128, 1024], F32, tag="sa",
                                         name=f"den{jj}_{hp}")
                    for (p0, p1) in ((0, 512), (512, 1024)):
                        nc.tensor.matmul(den_bc[0:64, p0:p1],
                                         ones_col[64:65, 0:64],
                                         da[64:65, p0:p1], start=True, stop=True)
                        nc.tensor.matmul(den_bc[64:128, p0:p1],
                                         ones_col[64:65, 0:64],
                                         db[64:65, p0:p1], start=True, stop=True)
                    nc.vector.reciprocal_approx_fast(out=rab, in_=den_bc)
                    if dbg is not None:
                        idx = jj * 4 + hp
                        nc.sync.dma_start(
                            dbg["rab"][idx * 128:(idx + 1) * 128, :], rab)
                    nc.vector.tensor_mul(ys[hp][:, cols], ys[hp][:, cols], rab)

        if dbg is not None:
            for j in range(4):
                nc.sync.dma_start(dbg["y"][j * 128:(j + 1) * 128, :], ys[j])

        # ---- projection: out[T, C] partial = y @ Wp ----
        with ExitStack() as ph3:
            wp_pool = ph3.enter_context(tc.tile_pool(name="wpp", bufs=1))
            wps = [wp_pool.tile([128, C], F32, tag=f"wp{kk}", name=f"wp{kk}")
                   for kk in range(4)]
            for kk in range(4):
                nc.sync.dma_start(wps[kk], wp[kk * 128:(kk + 1) * 128, :])
            proj_ps = ph3.enter_context(tc.tile_pool(name="projps", bufs=1,
                                                     space="PSUM"))
            o_pool = ph3.enter_context(tc.tile_pool(name="opool", bufs=3))
            for t in range(16):
                po = proj_ps.tile([128, 1024], F32, tag="po", bufs=2,
                                  name=f"po{t}")
                tcols = slice(t * 128, (t + 1) * 128)
                for kk in range(4):
                    for nn in range(2):
                        nc.tensor.matmul(po[:, nn * 512:(nn + 1) * 512],
                                         ys[kk][:, tcols],
                                         wps[kk][:, nn * 512:(nn + 1) * 512],
                                         start=(kk == 0), stop=(kk == 3))
                ot = o_pool.tile([128, 1024], F32, tag="ot", name=f"ot{t}")
                nc.vector.tensor_copy(ot, po)
                nc.sync.dma_start(out[tcols, :], ot)


def _build(debug=False):
    nc = bacc.Bacc("TRN2", target_bir_lowering=False, debug=False,
                   enable_asserts=False, num_devices=8)
    xt = nc.dram_tensor("xt", [C, T], F32, kind="ExternalInput").ap()
    wqk = nc.dram_tensor("wqk", [C, 2 * CS], F32, kind="ExternalInput").ap()
    bqk = nc.dram_tensor("bqk", [2 * CS], F32, kind="ExternalInput").ap()
    wv = nc.dram_tensor("wv", [C, CS], F32, kind="ExternalInput").ap()
    bv = nc.dram_tensor("bv", [CS], F32, kind="ExternalInput").ap()
    wp = nc.dram_tensor("wp", [CS, C], F32, kind="ExternalInput").ap()
    mask = nc.dram_tensor("mask", [128, 128], F32, kind="ExternalInput").ap()
    out = nc.dram_tensor("out", [T, C], F32, kind="ExternalOutput").ap()
    dbg = None
    if debug:
        dbg = {
            "qt": nc.dram_tensor("dbg_qt", [CS, T], F32, kind="ExternalOutput").ap(),
            "kt": nc.dram_tensor("dbg_kt", [CS, T], F32, kind="ExternalOutput").ap(),
            "v": nc.dram_tensor("dbg_v", [16 * 128, NPAIR * 130], F32,
                                kind="ExternalOutput").ap(),
            "y": nc.dram_tensor("dbg_y", [CS, T], F32, kind="ExternalOutput").ap(),
            "p": nc.dram_tensor("dbg_p", [256, 1024], F32,
                                kind="ExternalOutput").ap(),
            "rab": nc.dram_tensor("dbg_rab", [8 * 128, 1024], F32,
                                  kind="ExternalOutput").ap(),
        }
    with tile.TileContext(nc) as tc:
        _emit(tc, xt, wqk, bqk, wv, bv, wp, mask, out, dbg=dbg)
    nc.compile()
    return nc


_NC = None


def _get_nc():
    global _NC
    if _NC is None:
        _NC = _build()
    return _NC


def make_in_maps(x, w_attn, b_attn, w_proj):
    x = np.ascontiguousarray(np.asarray(x, np.float32))
    w_attn = np.asarray(w_attn, np.float32)
    b_attn = np.asarray(b_attn, np.float32)
    w_proj = np.asarray(w_proj, np.float32)
    mask = (np.arange(128)[None, :] >= np.arange(128)[:, None]).astype(np.float32)
    in_maps = []
    for c in range(8):
        b, hg = divmod(c, 2)
        q0 = hg * CS
        wqk_c = np.concatenate(
            [w_attn[:, q0:q0 + CS], w_attn[:, C + q0:C + q0 + CS]], axis=1)
        bqk_c = np.concatenate([b_attn[q0:q0 + CS], b_attn[C + q0:C + q0 + CS]])
        in_maps.append(dict(
            xt=np.ascontiguousarray(x[b].T),
            wqk=np.ascontiguousarray(wqk_c),
            bqk=np.ascontiguousarray(bqk_c),
            wv=np.ascontiguousarray(w_attn[:, 2 * C + q0:2 * C + q0 + CS]),
            bv=np.ascontiguousarray(b_attn[2 * C + q0:2 * C + q0 + CS]),
            wp=np.ascontiguousarray(w_proj[q0:q0 + CS, :]),
            mask=mask,
        ))
    return in_maps


def gather(results, b_proj):
    b_proj = np.asarray(b_proj, np.float32)
    y = np.empty((4, T, C), np.float32)
    for b in range(4):
        y[b] = results[2 * b]["out"] + results[2 * b + 1]["out"] + b_proj[None, :]
    return y


def run_spmd(in_maps, **kw):
    return run_bass_kernel_spmd(_get_nc(), in_maps, core_ids=list(range(8)), **kw)


def kernel(x, w_attn, b_attn, w_proj, b_proj):
    res = run_spmd(make_in_maps(x, w_attn, b_attn, w_proj))
    return gather(res.results, b_proj)
